# revision 35
# baseline (speedup 1.0000x reference)
"""ChebConv layer (K=3) on 8 TRN2 NeuronCores, data-parallel over batch.

Math:  out = relu(sum_k T_k(L) @ x @ Theta_k),  L = 2A/lambda - I,
       T_0=I, T_1=L, T_2=2L^2-I.
Re-expanded in powers of S = (2/lambda)*A (no identity terms on device):
       out = relu(Z_A + S @ (Z_B + S @ Z_C))
       Z_C = x@(2*Th2), Z_B = x@(Th1 - 4*Th2), Z_A = x@(Th0 - Th1 + Th2)

The tiny feature transforms (x@Theta, ~5% of FLOPs) fold into host prep;
the device runs the two dense 1024-deep graph aggregations (~95% of
FLOPs) as fp8 DoubleRow matmuls:
  H1 (normal):     U[n,to]    = S-pieces (stationary) @ Z_C (moving) + Z_B
  H2 (transposed): O^T[to,n]  = U-pieces (stationary) @ S^T (moving) + Z_A^T
The transposed H2 reuses the same SBUF-resident S^T tiles as H1 and makes
its output layout match the host-prepped Z_A^T, so no on-device transposes
exist anywhere.  Output leaves as bf16 O^T pieces; host transposes/upcasts.

Scales: st = S^T*4096 (fp8), zc/zb/u stored *4 (fp8/bf16/fp8), za exact.
Combines: u = ps1/4096 + zb, o = ps2/16384 + za.  zb stays bf16: with an
fp8 zb, the later fp8 u write is a double rounding of u ~= zb + small
S@Z_C increment, which mangles the increment (4x error inflation).
to-index = t*64+o throughout (t-major).
"""

import os
import sys

import numpy as np

sys.path.insert(0, "/opt/trn_rl_repo")

B, T, N, FIN = 32, 12, 1024, 64
K, OUT_F = 3, 64
NCORES = 8
BPC = B // NCORES          # batches per core
NCHUNK = N // 128          # 8 node chunks
TP = T // 2                # 6 output to-chunks (t-pairs)
TO = T * OUT_F             # 768 flattened (t, out_feature) columns
SSCALE = 4096.0            # host pre-scale of S into fp8e4m3 range
ZS = 4.0                   # Z_C / U fp8 storage scale
FP8MAX = 240.0             # TRN fp8e4 saturates to Inf beyond this

_CACHE = {}
LAST_RESULT = None


def _build_nc():
    import concourse.bacc as bacc
    import concourse.mybir as mybir
    import concourse.tile as tile
    from contextlib import ExitStack

    dt = mybir.dt
    f32, bf16, fp8 = dt.float32, dt.bfloat16, dt.float8e4
    DR = mybir.MatmulPerfMode.DoubleRow
    ACT = mybir.ActivationFunctionType

    nc = bacc.Bacc()
    st_d = nc.declare_dram_parameter("st", [BPC, N, N], fp8, isOutput=False)
    zc_d = nc.declare_dram_parameter("zc", [BPC, 128, NCHUNK, TO], fp8, isOutput=False)
    zb_d = nc.declare_dram_parameter("zb", [BPC, 128, NCHUNK, TO], bf16, isOutput=False)
    za_d = nc.declare_dram_parameter("za", [BPC, 128, TP, N], bf16, isOutput=False)
    out_d = nc.declare_dram_parameter("out", [BPC, TP, 128, N], bf16, isOutput=True)

    with tile.TileContext(nc) as tc, ExitStack() as ctx:
        st_pool = ctx.enter_context(tc.tile_pool(name="stp", bufs=4))
        zc_pool = ctx.enter_context(tc.tile_pool(name="zcp", bufs=4))
        zb_pool = ctx.enter_context(tc.tile_pool(name="zbp", bufs=4))
        za_pool = ctx.enter_context(tc.tile_pool(name="zap", bufs=4))
        u_pool = ctx.enter_context(tc.tile_pool(name="up", bufs=3))
        o_pool = ctx.enter_context(tc.tile_pool(name="op", bufs=3))
        ps_pool = ctx.enter_context(tc.tile_pool(name="psp", bufs=3, space="PSUM"))

        st_tiles, zc_tiles, zb_tiles, za_tiles, u_tiles = {}, {}, {}, {}, {}

        def emit_loads(b):
            if b in st_tiles:
                return
            st_t = st_pool.tile([128, NCHUNK * N], fp8, name=f"st_{b}", tag="st")
            st3 = st_t.rearrange("p (k n) -> p k n", n=N)
            sd3 = st_d[b].rearrange("(k p) n -> p k n", p=128)
            for k in range(NCHUNK):
                nc.sync.dma_start(out=st3[:, k : k + 1], in_=sd3[:, k : k + 1])
            zc_t = zc_pool.tile([128, NCHUNK, TO], fp8, name=f"zc_{b}", tag="zc")
            for k in range(0, NCHUNK, 2):
                nc.sync.dma_start(
                    out=zc_t[:, k : k + 2], in_=zc_d[b, :, k : k + 2]
                )
            zb_t = zb_pool.tile([128, NCHUNK, TO], bf16, name=f"zb_{b}", tag="zb")
            for k in range(0, NCHUNK, 4):
                nc.scalar.dma_start(
                    out=zb_t[:, k : k + 4], in_=zb_d[b, :, k : k + 4]
                )
            za_t = za_pool.tile([128, TP, N], bf16, name=f"za_{b}", tag="za")
            for k in range(0, TP, 3):
                nc.scalar.dma_start(
                    out=za_t[:, k : k + 3], in_=za_d[b, :, k : k + 3]
                )
            st_tiles[b] = st_t
            zc_tiles[b], zb_tiles[b], za_tiles[b] = zc_t, zb_t, za_t

        # ---- hop1 (normal): U[n-chunk c, to] = S@Z_C + Z_B ----
        def h1_group(b, c):
            st3 = st_tiles[b].rearrange("p (k n) -> p k n", n=N)
            zc_t, zb_t = zc_tiles[b], zb_tiles[b]
            if b not in u_tiles:
                u_tiles[b] = u_pool.tile(
                    [128, NCHUNK, TO], fp8, name=f"u_{b}", tag="u"
                )
            u3 = u_tiles[b]
            p1a = ps_pool.tile([128, 384], f32, name=f"p1a_{b}_{c}", tag="p1a")
            p1b = ps_pool.tile([128, 384], f32, name=f"p1b_{b}_{c}", tag="p1b", bufs=2)
            for q in range(NCHUNK // 2):
                lw = st3[:, 2 * q : 2 * q + 2, c * 128 : (c + 1) * 128]
                nc.tensor.matmul(
                    p1a[:],
                    lw,
                    zc_t[:, 2 * q : 2 * q + 2, 0:384],
                    start=(q == 0),
                    stop=(q == NCHUNK // 2 - 1),
                    perf_mode=DR,
                )
                nc.tensor.matmul(
                    p1b[:],
                    lw,
                    zc_t[:, 2 * q : 2 * q + 2, 384:768],
                    start=(q == 0),
                    stop=(q == NCHUNK // 2 - 1),
                    perf_mode=DR,
                )
            nc.vector.scalar_tensor_tensor(
                u3[:, c, 0:384],
                p1a[:],
                1.0 / 4096.0,
                zb_t[:, c, 0:384],
                op0=mybir.AluOpType.mult,
                op1=mybir.AluOpType.add,
            )
            nc.vector.scalar_tensor_tensor(
                u3[:, c, 384:768],
                p1b[:],
                1.0 / 4096.0,
                zb_t[:, c, 384:768],
                op0=mybir.AluOpType.mult,
                op1=mybir.AluOpType.add,
            )

        # ---- hop2 (transposed): O^T[to-chunk j, n] = U^T@S^T + Z_A^T ----
        def h2_group(b, j):
            st3 = st_tiles[b].rearrange("p (k n) -> p k n", n=N)
            u3, za = u_tiles[b], za_tiles[b]
            o_t = o_pool.tile([128, N], bf16, name=f"o_{b}_{j}", tag="o")
            for h in range(2):
                ps2 = ps_pool.tile(
                    [128, 512], f32, name=f"ps2_{b}_{j}_{h}", tag="ps2"
                )
                for q in range(NCHUNK // 2):
                    nc.tensor.matmul(
                        ps2[:],
                        u3[:, 2 * q : 2 * q + 2, j * 128 : (j + 1) * 128],
                        st3[:, 2 * q : 2 * q + 2, h * 512 : (h + 1) * 512],
                        start=(q == 0),
                        stop=(q == NCHUNK // 2 - 1),
                        perf_mode=DR,
                    )
                nc.vector.scalar_tensor_tensor(
                    o_t[:, h * 512 : (h + 1) * 512],
                    ps2[:],
                    1.0 / 16384.0,
                    za[:, j, h * 512 : (h + 1) * 512],
                    op0=mybir.AluOpType.mult,
                    op1=mybir.AluOpType.add,
                )
            nc.scalar.activation(o_t[:], o_t[:], ACT.Relu)
            nc.sync.dma_start(out=out_d[b, j], in_=o_t[:])

        # ---- software pipeline: step b runs H2(b-1) and H1(b) ----
        for b in range(BPC):
            emit_loads(b)
        for b in range(BPC):
            for c in range(NCHUNK):
                h1_group(b, c)
                if b > 0 and c < TP:
                    h2_group(b - 1, c)
        for j in range(TP):
            h2_group(BPC - 1, j)
    nc.compile()
    return nc


def _get_nc():
    if "nc" not in _CACHE:
        _CACHE["nc"] = _build_nc()
    return _CACHE["nc"]


def _to_fp8(a):
    import ml_dtypes

    return np.clip(a, -FP8MAX, FP8MAX).astype(ml_dtypes.float8_e4m3)


def _prep_core(x_c, A_c, thC, thB, thA):
    import ml_dtypes

    lam = np.maximum(A_c.sum(axis=-1).max(axis=-1), 1.0)  # [BPC]
    sT = A_c.transpose(0, 2, 1) * (2.0 / lam)[:, None, None]
    st = np.ascontiguousarray(_to_fp8(sT * SSCALE))

    xf = x_c.reshape(-1, FIN)
    zC = (xf @ (thC * ZS)).reshape(BPC, T, N, OUT_F)
    zB = (xf @ (thB * ZS)).reshape(BPC, T, N, OUT_F)
    zA = (xf @ thA).reshape(BPC, T, N, OUT_F)
    # zc/zb[b, p, c, t*64+o] = Z[b, t, n=c*128+p, o]*ZS
    zc = np.ascontiguousarray(
        _to_fp8(zC.reshape(BPC, T, NCHUNK, 128, OUT_F).transpose(0, 3, 2, 1, 4)
                .reshape(BPC, 128, NCHUNK, TO))
    )
    zb = np.ascontiguousarray(
        zB.reshape(BPC, T, NCHUNK, 128, OUT_F).transpose(0, 3, 2, 1, 4)
        .reshape(BPC, 128, NCHUNK, TO).astype(ml_dtypes.bfloat16)
    )
    # za[b, par*64+o, tp, n] = Z_A[b, 2tp+par, n, o]
    za = np.ascontiguousarray(
        zA.reshape(BPC, TP, 2, N, OUT_F).transpose(0, 2, 4, 1, 3)
        .reshape(BPC, 128, TP, N).astype(ml_dtypes.bfloat16)
    )
    return {"st": st, "zc": zc, "zb": zb, "za": za}


def kernel(x, A, Theta):
    global LAST_RESULT
    from concourse.bass_utils import run_bass_kernel_spmd

    x = np.asarray(x, dtype=np.float32)
    A = np.asarray(A, dtype=np.float32)
    Theta = np.asarray(Theta, dtype=np.float32)

    T0, T1, T2 = Theta[0], Theta[1], Theta[2]
    thC, thB, thA = 2.0 * T2, T1 - 4.0 * T2, T0 - T1 + T2

    nc = _get_nc()
    in_maps = [
        _prep_core(x[c * BPC : (c + 1) * BPC], A[c * BPC : (c + 1) * BPC],
                   thC, thB, thA)
        for c in range(NCORES)
    ]
    trace = bool(int(os.environ.get("CHEB_TRACE", "0")))
    res = run_bass_kernel_spmd(nc, in_maps, list(range(NCORES)), trace=trace)
    LAST_RESULT = res

    outs = []
    for c in range(NCORES):
        od = np.asarray(res.results[c]["out"])  # [BPC, 6, 128, 1024] bf16
        # od[b, j, par*64+o, n] = out[b, 2j+par, n, o]
        r = (
            od.astype(np.float32)
            .reshape(BPC, TP, 2, OUT_F, N)   # b, j, par, o, n
            .transpose(0, 1, 2, 4, 3)        # b, j, par, n, o
            .reshape(BPC, T, N, OUT_F)
        )
        outs.append(r)
    return np.ascontiguousarray(np.concatenate(outs, axis=0).astype(np.float32))


# revision 39
# speedup vs baseline: 1.0711x; 1.0711x over previous
"""ChebConv layer (K=3) on 8 TRN2 NeuronCores, data-parallel over batch.

Math:  out = relu(sum_k T_k(L) @ x @ Theta_k),  L = 2A/lambda - I,
       T_0=I, T_1=L, T_2=2L^2-I.
Re-expanded in powers of S = (2/lambda)*A (no identity terms on device):
       out = relu(Z_A + S @ (Z_B + S @ Z_C))
       Z_C = x@(2*Th2), Z_B = x@(Th1 - 4*Th2), Z_A = x@(Th0 - Th1 + Th2)

The tiny feature transforms (x@Theta, ~5% of FLOPs) fold into host prep;
the device runs the two dense 1024-deep graph aggregations (~95% of
FLOPs) as fp8 DoubleRow matmuls:
  H1 (normal):     U[n,to]    = S-pieces (stationary) @ Z_C (moving) + Z_B
  H2 (transposed): O^T[to,n]  = U-pieces (stationary) @ S^T (moving) + Z_A^T
The transposed H2 reuses the same SBUF-resident S^T tiles as H1 and makes
its output layout match the host-prepped Z_A^T, so no on-device transposes
exist anywhere.  Output leaves as bf16 O^T pieces; host transposes/upcasts.

Scales: st = S^T*4096 (fp8), zc/zb/u stored *4 (fp8/bf16/fp8), za exact.
Combines: u = ps1/4096 + zb, o = ps2/16384 + za.  zb stays bf16: with an
fp8 zb, the later fp8 u write is a double rounding of u ~= zb + small
S@Z_C increment, which mangles the increment (4x error inflation).
to-index = t*64+o throughout (t-major).
"""

import os
import sys

import numpy as np

sys.path.insert(0, "/opt/trn_rl_repo")

B, T, N, FIN = 32, 12, 1024, 64
K, OUT_F = 3, 64
NCORES = 8
BPC = B // NCORES          # batches per core
NCHUNK = N // 128          # 8 node chunks
TP = T // 2                # 6 output to-chunks (t-pairs)
TO = T * OUT_F             # 768 flattened (t, out_feature) columns
SSCALE = 4096.0            # host pre-scale of S into fp8e4m3 range
ZS = 4.0                   # Z_C / U fp8 storage scale
FP8MAX = 240.0             # TRN fp8e4 saturates to Inf beyond this

_CACHE = {}
LAST_RESULT = None


def _build_nc():
    import concourse.bacc as bacc
    import concourse.mybir as mybir
    import concourse.tile as tile
    from contextlib import ExitStack

    dt = mybir.dt
    f32, bf16, fp8 = dt.float32, dt.bfloat16, dt.float8e4
    DR = mybir.MatmulPerfMode.DoubleRow
    ACT = mybir.ActivationFunctionType

    nc = bacc.Bacc()
    st_d = nc.declare_dram_parameter("st", [BPC, N, N], fp8, isOutput=False)
    zc_d = nc.declare_dram_parameter("zc", [BPC, 128, NCHUNK, TO], fp8, isOutput=False)
    zb_d = nc.declare_dram_parameter("zb", [BPC, 128, NCHUNK, TO], bf16, isOutput=False)
    za_d = nc.declare_dram_parameter("za", [BPC, 128, TP, N], bf16, isOutput=False)
    out_d = nc.declare_dram_parameter("out", [BPC, TP, 128, N], bf16, isOutput=True)

    with tile.TileContext(nc) as tc, ExitStack() as ctx:
        st_pool = ctx.enter_context(tc.tile_pool(name="stp", bufs=4))
        zc_pool = ctx.enter_context(tc.tile_pool(name="zcp", bufs=4))
        zb_pool = ctx.enter_context(tc.tile_pool(name="zbp", bufs=4))
        za_pool = ctx.enter_context(tc.tile_pool(name="zap", bufs=4))
        u_pool = ctx.enter_context(tc.tile_pool(name="up", bufs=3))
        o_pool = ctx.enter_context(tc.tile_pool(name="op", bufs=3))
        ps_pool = ctx.enter_context(tc.tile_pool(name="psp", bufs=3, space="PSUM"))

        st_tiles, zc_tiles, zb_tiles, za_tiles, u_tiles = {}, {}, {}, {}, {}

        def emit_loads(b):
            if b in st_tiles:
                return
            st_t = st_pool.tile([128, NCHUNK * N], fp8, name=f"st_{b}", tag="st")
            st3 = st_t.rearrange("p (k n) -> p k n", n=N)
            sd3 = st_d[b].rearrange("(k p) n -> p k n", p=128)
            for k in range(0, NCHUNK, 2):
                eng = nc.scalar if (b == 0 and (k // 2) % 2 == 1) else nc.sync
                eng.dma_start(out=st3[:, k : k + 2], in_=sd3[:, k : k + 2])
            zc_t = zc_pool.tile([128, NCHUNK, TO], fp8, name=f"zc_{b}", tag="zc")
            for k in range(0, NCHUNK, 4):
                eng = nc.scalar if (b == 0 and k >= 4) else nc.sync
                eng.dma_start(
                    out=zc_t[:, k : k + 4], in_=zc_d[b, :, k : k + 4]
                )
            zb_t = zb_pool.tile([128, NCHUNK, TO], bf16, name=f"zb_{b}", tag="zb")
            for k in range(0, NCHUNK, 4):
                nc.scalar.dma_start(
                    out=zb_t[:, k : k + 4], in_=zb_d[b, :, k : k + 4]
                )
            za_t = za_pool.tile([128, TP, N], bf16, name=f"za_{b}", tag="za")
            for k in range(0, TP, 3):
                nc.scalar.dma_start(
                    out=za_t[:, k : k + 3], in_=za_d[b, :, k : k + 3]
                )
            st_tiles[b] = st_t
            zc_tiles[b], zb_tiles[b], za_tiles[b] = zc_t, zb_t, za_t

        # ---- hop1 (normal): U[n-chunk c, to] = S@Z_C + Z_B ----
        def h1_group(b, c):
            st3 = st_tiles[b].rearrange("p (k n) -> p k n", n=N)
            zc_t, zb_t = zc_tiles[b], zb_tiles[b]
            if b not in u_tiles:
                u_tiles[b] = u_pool.tile(
                    [128, NCHUNK, TO], fp8, name=f"u_{b}", tag="u"
                )
            u3 = u_tiles[b]
            p1a = ps_pool.tile([128, 384], f32, name=f"p1a_{b}_{c}", tag="p1a")
            p1b = ps_pool.tile([128, 384], f32, name=f"p1b_{b}_{c}", tag="p1b", bufs=2)
            for q in range(NCHUNK // 2):
                lw = st3[:, 2 * q : 2 * q + 2, c * 128 : (c + 1) * 128]
                nc.tensor.matmul(
                    p1a[:],
                    lw,
                    zc_t[:, 2 * q : 2 * q + 2, 0:384],
                    start=(q == 0),
                    stop=(q == NCHUNK // 2 - 1),
                    perf_mode=DR,
                )
                nc.tensor.matmul(
                    p1b[:],
                    lw,
                    zc_t[:, 2 * q : 2 * q + 2, 384:768],
                    start=(q == 0),
                    stop=(q == NCHUNK // 2 - 1),
                    perf_mode=DR,
                )
            nc.vector.scalar_tensor_tensor(
                u3[:, c, 0:384],
                p1a[:],
                1.0 / 4096.0,
                zb_t[:, c, 0:384],
                op0=mybir.AluOpType.mult,
                op1=mybir.AluOpType.add,
            )
            nc.vector.scalar_tensor_tensor(
                u3[:, c, 384:768],
                p1b[:],
                1.0 / 4096.0,
                zb_t[:, c, 384:768],
                op0=mybir.AluOpType.mult,
                op1=mybir.AluOpType.add,
            )

        # ---- hop2 (transposed): O^T[to-chunk j, n] = U^T@S^T + Z_A^T ----
        def h2_group(b, j, fine_tail=False):
            st3 = st_tiles[b].rearrange("p (k n) -> p k n", n=N)
            u3, za = u_tiles[b], za_tiles[b]
            o_t = o_pool.tile([128, N], bf16, name=f"o_{b}_{j}", tag="o")
            for h in range(2):
                ps2 = ps_pool.tile(
                    [128, 512], f32, name=f"ps2_{b}_{j}_{h}", tag="ps2"
                )
                for q in range(NCHUNK // 2):
                    nc.tensor.matmul(
                        ps2[:],
                        u3[:, 2 * q : 2 * q + 2, j * 128 : (j + 1) * 128],
                        st3[:, 2 * q : 2 * q + 2, h * 512 : (h + 1) * 512],
                        start=(q == 0),
                        stop=(q == NCHUNK // 2 - 1),
                        perf_mode=DR,
                    )
                nc.vector.scalar_tensor_tensor(
                    o_t[:, h * 512 : (h + 1) * 512],
                    ps2[:],
                    1.0 / 16384.0,
                    za[:, j, h * 512 : (h + 1) * 512],
                    op0=mybir.AluOpType.mult,
                    op1=mybir.AluOpType.add,
                )
                if fine_tail:
                    sl = slice(h * 512, (h + 1) * 512)
                    nc.scalar.activation(o_t[:, sl], o_t[:, sl], ACT.Relu)
                    nc.sync.dma_start(out=out_d[b, j, :, sl], in_=o_t[:, sl])
            if not fine_tail:
                nc.scalar.activation(o_t[:], o_t[:], ACT.Relu)
                nc.sync.dma_start(out=out_d[b, j], in_=o_t[:])

        # ---- software pipeline: step b runs H2(b-1) and H1(b) ----
        for b in range(BPC):
            emit_loads(b)
        for b in range(BPC):
            for c in range(NCHUNK):
                h1_group(b, c)
                if b > 0 and c < TP:
                    h2_group(b - 1, c)
        for j in range(TP):
            h2_group(BPC - 1, j, fine_tail=(j == TP - 1))
    nc.compile()
    return nc


def _get_nc():
    if "nc" not in _CACHE:
        _CACHE["nc"] = _build_nc()
    return _CACHE["nc"]


def _to_fp8(a):
    import ml_dtypes

    return np.clip(a, -FP8MAX, FP8MAX).astype(ml_dtypes.float8_e4m3)


def _prep_core(x_c, A_c, thC, thB, thA):
    import ml_dtypes

    lam = np.maximum(A_c.sum(axis=-1).max(axis=-1), 1.0)  # [BPC]
    sT = A_c.transpose(0, 2, 1) * (2.0 / lam)[:, None, None]
    st = np.ascontiguousarray(_to_fp8(sT * SSCALE))

    xf = x_c.reshape(-1, FIN)
    zC = (xf @ (thC * ZS)).reshape(BPC, T, N, OUT_F)
    zB = (xf @ (thB * ZS)).reshape(BPC, T, N, OUT_F)
    zA = (xf @ thA).reshape(BPC, T, N, OUT_F)
    # zc/zb[b, p, c, t*64+o] = Z[b, t, n=c*128+p, o]*ZS
    zc = np.ascontiguousarray(
        _to_fp8(zC.reshape(BPC, T, NCHUNK, 128, OUT_F).transpose(0, 3, 2, 1, 4)
                .reshape(BPC, 128, NCHUNK, TO))
    )
    zb = np.ascontiguousarray(
        zB.reshape(BPC, T, NCHUNK, 128, OUT_F).transpose(0, 3, 2, 1, 4)
        .reshape(BPC, 128, NCHUNK, TO).astype(ml_dtypes.bfloat16)
    )
    # za[b, par*64+o, tp, n] = Z_A[b, 2tp+par, n, o]
    za = np.ascontiguousarray(
        zA.reshape(BPC, TP, 2, N, OUT_F).transpose(0, 2, 4, 1, 3)
        .reshape(BPC, 128, TP, N).astype(ml_dtypes.bfloat16)
    )
    return {"st": st, "zc": zc, "zb": zb, "za": za}


def kernel(x, A, Theta):
    global LAST_RESULT
    from concourse.bass_utils import run_bass_kernel_spmd

    x = np.asarray(x, dtype=np.float32)
    A = np.asarray(A, dtype=np.float32)
    Theta = np.asarray(Theta, dtype=np.float32)

    T0, T1, T2 = Theta[0], Theta[1], Theta[2]
    thC, thB, thA = 2.0 * T2, T1 - 4.0 * T2, T0 - T1 + T2

    nc = _get_nc()
    in_maps = [
        _prep_core(x[c * BPC : (c + 1) * BPC], A[c * BPC : (c + 1) * BPC],
                   thC, thB, thA)
        for c in range(NCORES)
    ]
    trace = bool(int(os.environ.get("CHEB_TRACE", "0")))
    res = run_bass_kernel_spmd(nc, in_maps, list(range(NCORES)), trace=trace)
    LAST_RESULT = res

    outs = []
    for c in range(NCORES):
        od = np.asarray(res.results[c]["out"])  # [BPC, 6, 128, 1024] bf16
        # od[b, j, par*64+o, n] = out[b, 2j+par, n, o]
        r = (
            od.astype(np.float32)
            .reshape(BPC, TP, 2, OUT_F, N)   # b, j, par, o, n
            .transpose(0, 1, 2, 4, 3)        # b, j, par, n, o
            .reshape(BPC, T, N, OUT_F)
        )
        outs.append(r)
    return np.ascontiguousarray(np.concatenate(outs, axis=0).astype(np.float32))
